# revision 27
# baseline (speedup 1.0000x reference)
"""GaussianImage splat kernel for 8 Trainium2 NeuronCores.

Math (matches reference.py):
    cov      = R(theta) S S^T R(theta)^T per gaussian
    q(n, p)  = (p - mu_n)^T cov_n^{-1} (p - mu_n)
    prob     = exp(-q/2) / (2*pi*sqrt(det))
    splat    = prob / max(prob)           (global max over all n, p)
    img      = einsum('nhw,nc->hwc', splat, rgb*alpha)
    out      = sigmoid(img)

Device formulation:
    r(n, p)  = G[n] . F[p]   with F = [1, x', y', x'^2, x'y', y'^2] (centered
               monomials, x' = x - 0.5) and G the folded per-gaussian
               coefficients including log norm -> prob = exp(r).
    MM1 (PE, row-tiled 4x128-gaussian chunks), exp on ACT (PSUM->SBUF bf16),
    running elementwise max on DVE, MM2 (PE) contracts gaussians against
    w = rgb*alpha, 4 image tiles packed per PSUM bank at partition bases
    {0,32,64,96}, DVE evacuates, DMA repacks to pixel-major, AllReduce(max)
    over the 8 cores, then sigmoid via tanh (same ACT table set as exp).

Sharding: pixels split across the 8 cores (each core: all 512 gaussians x
32768 pixels); only a 512-byte all-reduce(max) collective is needed.
"""

import os
import sys
import numpy as np

for _p in ("/opt/trn_rl_repo", "/root/.axon_site/_ro/trn_rl_repo"):
    if os.path.isdir(_p) and _p not in sys.path:
        sys.path.insert(0, _p)

import ml_dtypes  # noqa: E402

def _ensure_ntff_hook():
    """antenv.axon_hooks is missing in this image; shim it and register the
    ctypes NTFF profiler from trn_agent_boot so trace=True works."""
    import types

    try:
        import antenv.axon_hooks  # noqa: F401
        return
    except ImportError:
        pass
    import antenv

    mod = types.ModuleType("antenv.axon_hooks")
    mod._hook = None

    def set_axon_ntff_profile_hook(h):
        mod._hook = h

    def get_axon_ntff_profile_hook():
        return mod._hook

    mod.set_axon_ntff_profile_hook = set_axon_ntff_profile_hook
    mod.get_axon_ntff_profile_hook = get_axon_ntff_profile_hook
    sys.modules["antenv.axon_hooks"] = mod
    antenv.axon_hooks = mod
    try:
        from trn_agent_boot.trn_boot import _ntff_profile_via_ctypes

        set_axon_ntff_profile_hook(
            _ntff_profile_via_ctypes("/opt/axon/libaxon_pjrt.so")
        )
    except Exception:
        pass


_ensure_ntff_hook()

import concourse.bass as bass  # noqa: E402
import concourse.tile as tile  # noqa: E402
from concourse import bass_isa, library_config, mybir  # noqa: E402
from concourse.tile import add_dep_helper  # noqa: E402
from concourse.bass_utils import run_bass_kernel_spmd  # noqa: E402

F32 = mybir.dt.float32
BF16 = mybir.dt.bfloat16

N_GAUSS = 512
H = 512
W = 512
N_CORES = 8
PX_CORE = H * W // N_CORES      # 32768 pixels per core
PT = 512                        # pixels per tile
NT = PX_CORE // PT              # 64 tiles per core
NCHUNK = 4                      # gaussian chunks of 128

# MM1 moving/stationary dtype: F32 is exact (4 cyc/row), float32r is fast
# (1 cyc/row) with relaxed precision. Toggle via env for experiments.
MM1_DT = {"f32": F32, "f32r": mybir.dt.float32r}[os.environ.get("MM1_DT", "f32")]

LAST_EXEC_NS = None
LAST_RESULTS = None


def _build_nc():
    nc = bass.Bass(num_devices=N_CORES)

    # Per-core inputs (host pre-sliced / pre-massaged).
    pix = nc.declare_dram_parameter("pix", [128, 512], F32, isOutput=False)
    gco = nc.declare_dram_parameter("gco", [128, 128], MM1_DT, isOutput=False)
    wts = nc.declare_dram_parameter("wts", [128, 128], BF16, isOutput=False)
    out = nc.declare_dram_parameter("out", [128, 768], F32, isOutput=True)

    with tile.TileContext(nc) as tc:
        with (
            tc.tile_pool(name="const", bufs=1) as cpool,
            tc.tile_pool(name="rps", bufs=3, space="PSUM") as rpool,
            tc.tile_pool(name="ips", bufs=2, space="PSUM") as ipool,
            tc.tile_pool(name="prob", bufs=3) as ppool,
            tc.tile_pool(name="evac", bufs=3) as epool,
            tc.tile_pool(name="dram", bufs=1, space="DRAM") as dpool,
        ):
            # ---- load inputs --------------------------------------------
            t0 = cpool.tile([128, 512], F32, tag="t0")       # centered pixels
            nc.sync.dma_start(t0[:], pix[:])
            g_sb = cpool.tile([128, 128], MM1_DT, tag="g")
            nc.sync.dma_start(g_sb[:], gco[:])
            w_sb = cpool.tile([128, 128], BF16, tag="w")
            nc.sync.dma_start(w_sb[:], wts[:])

            # ---- feature planes [128, 6*256] ----------------------------
            # partition p holds pixels 256p..256p+255; t0 is (x', y')
            # interleaved so x' = t0[:, 0::2], y' = t0[:, 1::2].
            # One consolidated tile: all 6 writers are DVE -> one sem.
            xs = t0[:, 0:512:2]
            ys = t0[:, 1:512:2]
            planes = cpool.tile([128, 1536], F32, tag="planes")
            nc.vector.memset(planes[:, 0:256], 1.0)
            nc.vector.tensor_copy(planes[:, 256:512], xs)
            nc.vector.tensor_copy(planes[:, 512:768], ys)
            nc.vector.tensor_mul(planes[:, 768:1024], xs, xs)
            nc.vector.tensor_mul(planes[:, 1024:1280], xs, ys)
            nc.vector.tensor_mul(planes[:, 1280:1536], ys, ys)

            # ---- F matrix [128, 32768]: rows 32c+k = feature k ----------
            # Bounce via DRAM so each fmat row-group has exactly ONE writer
            # DMA (matmul sync-wait slots are scarce). DRAM layout is
            # [a=128][k=6][b=256]; fmat row 32c+k col 256a+b.
            fbounce = dpool.tile([128, 1536], F32, tag="fb")
            nc.sync.dma_start(fbounce[:], planes[:])
            fb_flat = fbounce[:].rearrange("a b -> (a b)")
            fmat = cpool.tile([128, PX_CORE], MM1_DT, tag="fmat")
            for c in range(NCHUNK):
                base = 32 * c
                dst = fmat[base : base + 6, :].rearrange(
                    "k (a b) -> k a b", a=128
                )
                src = fb_flat.rearrange("(a k b) -> k a b", k=6, b=256)
                nc.sync.dma_start(dst, src)

            # ---- persistent accumulators --------------------------------
            maxbuf = cpool.tile([128, 2048], BF16, tag="maxb")
            nc.vector.memset(maxbuf[:], 0.0)
            scratch = dpool.tile([128, 768], F32, tag="scr")  # pixel-major img
            scr_flat = scratch[:].rearrange("a b -> (a b)")

            # ---- main loop ----------------------------------------------
            for t in range(NT):
                # MM1: r = G . F, row-tiled over 4 gaussian chunks.
                r0 = rpool.tile([128, 1024], F32, tag="r")
                r1 = rpool.tile([128, 1024], F32, tag="r")
                rhalves = (r0, r1)
                for c in range(NCHUNK):
                    rh = rhalves[c // 2]
                    off = 512 * (c % 2)
                    nc.tensor.matmul(
                        rh[:, off : off + 512],
                        lhsT=g_sb[32 * c : 32 * c + 6, :],
                        rhs=fmat[32 * c : 32 * c + 6, PT * t : PT * (t + 1)],
                        start=True,
                        stop=True,
                        tile_position=(32 * c, 0),
                    )

                # exp on ACT: PSUM fp32 -> SBUF bf16
                prob = ppool.tile([128, 2048], BF16, tag="prob")
                nc.scalar.activation(
                    prob[:, 0:1024], r0[:], mybir.ActivationFunctionType.Exp
                )
                nc.scalar.activation(
                    prob[:, 1024:2048], r1[:], mybir.ActivationFunctionType.Exp
                )

                # running max (DVE, bf16 2x mode)
                nc.vector.tensor_tensor(
                    maxbuf[:], maxbuf[:], prob[:], op=mybir.AluOpType.max
                )

                # MM2: img[3, 512] at partition base 32*(t%4), 4 tiles/bank
                j = t % 4
                u = t // 4
                if j == 0:
                    imgp = ipool.tile([128, 512], F32, tag="img")
                for c in range(NCHUNK):
                    # M padded to 32 (cols 3..31 of wts are zero) so the
                    # whole 32-partition group is written -> defined PSUM.
                    nc.tensor.matmul(
                        imgp[32 * j : 32 * j + 32, :],
                        lhsT=w_sb[:, 32 * c : 32 * c + 32],
                        rhs=prob[:, 512 * c : 512 * (c + 1)],
                        start=(c == 0),
                        stop=(c == 3),
                        tile_position=(0, 32 * j),
                    )
                if j == 3:
                    # evacuate 4 image tiles with one [128, 512] DVE copy
                    ev = epool.tile([128, 512], F32, tag="ev")
                    nc.vector.tensor_copy(ev[:], imgp[:])
                    # repack to pixel-major DRAM scratch: tile 4u+jj covers
                    # pixels 512*(4u+jj)+i; scratch flat idx 3*px + ch.
                    # SW DMA queue (gpsimd) -> one semaphore for all of them.
                    for jj in range(4):
                        tt = 4 * u + jj
                        dst = scr_flat[1536 * tt : 1536 * (tt + 1)].rearrange(
                            "(i c) -> c i", c=3
                        )
                        nc.gpsimd.dma_start(dst, ev[32 * jj : 32 * jj + 3, :])

            # ---- global max ---------------------------------------------
            # per-partition max -> DRAM roundtrip to gather onto one
            # partition -> free-dim reduce -> AllReduce(max) across cores.
            maxcol = cpool.tile([128, 1], F32, tag="maxcol")
            nc.vector.tensor_reduce(
                maxcol[:],
                maxbuf[:],
                axis=mybir.AxisListType.X,
                op=mybir.AluOpType.max,
            )
            d1 = dpool.tile([1, 128], F32, tag="d1")
            nc.sync.dma_start(
                d1[:].rearrange("a b -> (a b)"),
                maxcol[:].rearrange("a b -> (a b)"),
            )
            row = cpool.tile([1, 128], F32, tag="row")
            nc.sync.dma_start(row[:], d1[:])
            pl = cpool.tile([1, 1], F32, tag="pl")
            nc.vector.tensor_reduce(
                pl[:], row[:], axis=mybir.AxisListType.X, op=mybir.AluOpType.max
            )
            # replicate local max along free dim for the collective buffer
            ones_row = cpool.tile([1, 128], F32, tag="onesrow")
            nc.vector.memset(ones_row[:], 1.0)
            lrow = cpool.tile([1, 128], F32, tag="lrow")
            nc.vector.tensor_scalar_mul(lrow[:], ones_row[:], pl[:])
            cc_in = dpool.tile([1, 128], F32, tag="ccin")
            cc_out = dpool.tile([1, 128], F32, tag="ccout")
            nc.sync.dma_start(cc_in[:], lrow[:])
            nc.gpsimd.collective_compute(
                "AllReduce",
                mybir.AluOpType.max,
                replica_groups=[list(range(N_CORES))],
                ins=[cc_in[:].opt()],
                outs=[cc_out[:].opt()],
            )
            pmax = cpool.tile([1, 1], F32, tag="pmax")
            nc.sync.dma_start(pmax[0:1, 0:1], cc_out[0:1, 0:1])

            # s = 1 / (2 * pmax), broadcast to all 128 partitions via DRAM
            srec = cpool.tile([1, 1], F32, tag="srec")
            nc.vector.tensor_scalar_mul(srec[:], pmax[:], 2.0)
            nc.vector.reciprocal(srec[:], srec[:])
            srow = cpool.tile([1, 128], F32, tag="srow")
            nc.vector.tensor_scalar_mul(srow[:], ones_row[:], srec[:])
            d2 = dpool.tile([1, 128], F32, tag="d2")
            nc.sync.dma_start(d2[:], srow[:])
            sbc = cpool.tile([128, 1], F32, tag="sbc")
            nc.sync.dma_start(
                sbc[:].rearrange("a b -> (a b)"),
                d2[:].rearrange("a b -> (a b)"),
            )

            # ---- final pass: sigmoid(acc/pmax) = 0.5 + 0.5*tanh(acc*s) --
            fin = cpool.tile([128, 768], F32, tag="fin")
            nc.sync.dma_start(fin[:], scratch[:])
            nc.vector.tensor_scalar_mul(fin[:], fin[:], sbc[:])
            nc.scalar.activation(
                fin[:], fin[:], mybir.ActivationFunctionType.Tanh
            )
            nc.vector.tensor_scalar(
                fin[:],
                fin[:],
                0.5,
                0.5,
                op0=mybir.AluOpType.mult,
                op1=mybir.AluOpType.add,
            )
            nc.sync.dma_start(out[:], fin[:])

    _legalize_waits(nc)
    return nc


# walrus encodes sync waits into fixed ISA struct slots (fused matmuls /
# TT hold only ONE). Hoist excess waits onto same-engine NOPs spliced
# immediately before the instruction — semantically identical (the engine
# stalls at the NOP instead).
def _legalize_waits(nc, cap=1):
    for blk in nc.main_func.blocks:
        insts = blk.instructions
        out = []
        for ins in insts:
            si = ins.sync_info
            if si is not None and len(si.on_wait) > cap:
                waits = list(si.on_wait)
                excess, keep = waits[:-cap], waits[-cap:]
                for w in excess:
                    eng = nc.engines[ins.engine]
                    n = eng.nop(hint="wait_legalize")
                    tail = nc.main_func.blocks[-1].instructions
                    assert tail[-1] is n.ins
                    tail.pop()
                    n.ins.sync_info = mybir.SyncInfo(
                        on_wait=[w], on_update=[]
                    )
                    out.append(n.ins)
                si.on_wait = keep
            out.append(ins)
        insts[:] = out


def _host_prep(mean, alpha, scale, theta, rgb, pixels):
    """Fold gaussian params into matmul coefficients (float64 on host)."""
    mean = np.asarray(mean, np.float64)
    alpha = np.asarray(alpha, np.float64)
    scale = np.asarray(scale, np.float64)
    theta = np.asarray(theta, np.float64)
    rgb = np.asarray(rgb, np.float64)
    pixels = np.asarray(pixels, np.float32)

    two_pi = 2.0 * np.pi
    ta = two_pi * theta[:, 0]
    c, s = np.cos(ta), np.sin(ta)
    sx2 = scale[:, 0] ** 2
    sy2 = scale[:, 1] ** 2
    A = c * c * sx2 + s * s * sy2
    Bc = c * s * (sx2 - sy2)
    D = s * s * sx2 + c * c * sy2
    det = A * D - Bc * Bc
    i00 = D / det
    iBs = -2.0 * Bc / det          # inv01 + inv10
    i11 = A / det
    lognorm = -np.log(two_pi) - 0.5 * np.log(det)

    px0 = mean[:, 0, 0] - 0.5      # gaussian means in centered coords
    py0 = mean[:, 1, 0] - 0.5
    c_1 = (
        -0.5 * (i00 * px0 * px0 + iBs * px0 * py0 + i11 * py0 * py0) + lognorm
    )
    c_x = i00 * px0 + 0.5 * iBs * py0
    c_y = 0.5 * iBs * px0 + i11 * py0
    c_xx = -0.5 * i00
    c_xy = -0.5 * iBs
    c_yy = -0.5 * i11

    gco = np.zeros((128, 128), np.float32)
    coefs = np.stack([c_1, c_x, c_y, c_xx, c_xy, c_yy])  # [6, 512]
    for ch in range(NCHUNK):
        gco[32 * ch : 32 * ch + 6, :] = coefs[:, 128 * ch : 128 * (ch + 1)]

    w = (rgb * alpha).astype(ml_dtypes.bfloat16)          # [512, 3]
    wts = np.zeros((128, 128), ml_dtypes.bfloat16)
    for ch in range(NCHUNK):
        wts[:, 32 * ch : 32 * ch + 3] = w[128 * ch : 128 * (ch + 1), :]

    # centered pixels, per-core slices in [128, 512] partition layout
    pc = (pixels.astype(np.float32) - np.float32(0.5)).reshape(-1)  # (H*W*2,)
    pix_cores = pc.reshape(N_CORES, 128, 512)
    return gco, wts, pix_cores


def make_in_maps(mean, alpha, scale, theta, rgb, pixels):
    gco, wts, pix_cores = _host_prep(mean, alpha, scale, theta, rgb, pixels)
    return [
        {"pix": np.ascontiguousarray(pix_cores[i]), "gco": gco, "wts": wts}
        for i in range(N_CORES)
    ]


def assemble(results):
    outs = [np.asarray(r["out"], np.float32).reshape(-1) for r in results]
    return np.concatenate(outs).reshape(H, W, 3)


def kernel(mean, alpha, scale, theta, rgb, pixels):
    global LAST_EXEC_NS, LAST_RESULTS
    in_maps = make_in_maps(mean, alpha, scale, theta, rgb, pixels)
    nc = _build_nc()
    trace = os.environ.get("KERNEL_TRACE", "0") == "1"
    res = run_bass_kernel_spmd(nc, in_maps, list(range(N_CORES)), trace=trace)
    LAST_EXEC_NS = res.exec_time_ns
    LAST_RESULTS = res
    return assemble(res.results)


# revision 33
# speedup vs baseline: 2.5108x; 2.5108x over previous
"""GaussianImage splat kernel for 8 Trainium2 NeuronCores.

Math (matches reference.py):
    cov      = R(theta) S S^T R(theta)^T per gaussian
    q(n, p)  = (p - mu_n)^T cov_n^{-1} (p - mu_n)
    prob     = exp(-q/2) / (2*pi*sqrt(det))
    splat    = prob / max(prob)           (global max over all n, p)
    img      = einsum('nhw,nc->hwc', splat, rgb*alpha)
    out      = sigmoid(img)

Device formulation:
    r(n, p)  = G[n] . F[p]   with F = [1, x', y', x'^2, x'y', y'^2] (centered
               monomials, x' = x - 0.5) and G the folded per-gaussian
               coefficients including log norm -> prob = exp(r).
    MM1 (PE, row-tiled 4x128-gaussian chunks), exp on ACT (PSUM->SBUF bf16),
    running elementwise max on DVE, MM2 (PE) contracts gaussians against
    w = rgb*alpha, 4 image tiles packed per PSUM bank at partition bases
    {0,32,64,96}, DVE evacuates, DMA repacks to pixel-major, AllReduce(max)
    over the 8 cores, then sigmoid via tanh (same ACT table set as exp).

Sharding: pixels split across the 8 cores (each core: all 512 gaussians x
32768 pixels); only a 512-byte all-reduce(max) collective is needed.
"""

import os
import sys
import numpy as np

for _p in ("/opt/trn_rl_repo", "/root/.axon_site/_ro/trn_rl_repo"):
    if os.path.isdir(_p) and _p not in sys.path:
        sys.path.insert(0, _p)

import ml_dtypes  # noqa: E402

def _ensure_ntff_hook():
    """antenv.axon_hooks is missing in this image; shim it and register the
    ctypes NTFF profiler from trn_agent_boot so trace=True works."""
    import types

    try:
        import antenv.axon_hooks  # noqa: F401
        return
    except ImportError:
        pass
    import antenv

    mod = types.ModuleType("antenv.axon_hooks")
    mod._hook = None

    def set_axon_ntff_profile_hook(h):
        mod._hook = h

    def get_axon_ntff_profile_hook():
        return mod._hook

    mod.set_axon_ntff_profile_hook = set_axon_ntff_profile_hook
    mod.get_axon_ntff_profile_hook = get_axon_ntff_profile_hook
    sys.modules["antenv.axon_hooks"] = mod
    antenv.axon_hooks = mod
    try:
        from trn_agent_boot.trn_boot import _ntff_profile_via_ctypes

        set_axon_ntff_profile_hook(
            _ntff_profile_via_ctypes("/opt/axon/libaxon_pjrt.so")
        )
    except Exception:
        pass


_ensure_ntff_hook()

import concourse.bass as bass  # noqa: E402
import concourse.tile as tile  # noqa: E402
from concourse import bass_isa, library_config, mybir  # noqa: E402
from concourse.tile import add_dep_helper  # noqa: E402
from concourse.bass_utils import run_bass_kernel_spmd  # noqa: E402

F32 = mybir.dt.float32
BF16 = mybir.dt.bfloat16

N_GAUSS = 512
H = 512
W = 512
N_CORES = 8
PX_CORE = H * W // N_CORES      # 32768 pixels per core
PT = 512                        # pixels per tile
NT = PX_CORE // PT              # 64 tiles per core
NCHUNK = 4                      # gaussian chunks of 128

# MM1 moving/stationary dtype: F32 is exact (4 cyc/row), float32r is fast
# (1 cyc/row) with relaxed precision. Toggle via env for experiments.
MM1_DT = {"f32": F32, "f32r": mybir.dt.float32r}[os.environ.get("MM1_DT", "f32")]

LAST_EXEC_NS = None
LAST_RESULTS = None


def _build_nc():
    nc = bass.Bass(num_devices=N_CORES)

    # Per-core inputs (host pre-sliced / pre-massaged).
    pix = nc.declare_dram_parameter("pix", [128, 512], F32, isOutput=False)
    gco = nc.declare_dram_parameter("gco", [128, 128], MM1_DT, isOutput=False)
    wts = nc.declare_dram_parameter("wts", [128, 128], BF16, isOutput=False)
    out = nc.declare_dram_parameter("out", [96, 1024], F32, isOutput=True)

    with tile.TileContext(nc) as tc:
        with (
            tc.tile_pool(name="const", bufs=1) as cpool,
            tc.tile_pool(name="rps", bufs=3, space="PSUM") as rpool,
            tc.tile_pool(name="ips", bufs=2, space="PSUM") as ipool,
            tc.tile_pool(name="prob", bufs=3) as ppool,
            tc.tile_pool(name="evac", bufs=3) as epool,
            tc.tile_pool(name="dram", bufs=1, space="DRAM") as dpool,
        ):
            # ---- load inputs --------------------------------------------
            t0 = cpool.tile([128, 512], F32, tag="t0")       # centered pixels
            nc.sync.dma_start(t0[:], pix[:])
            g_sb = cpool.tile([128, 128], MM1_DT, tag="g")
            nc.sync.dma_start(g_sb[:], gco[:])
            w_sb = cpool.tile([128, 128], BF16, tag="w")
            nc.sync.dma_start(w_sb[:], wts[:])

            # ---- feature planes [128, 6*256] ----------------------------
            # partition p holds pixels 256p..256p+255; t0 is (x', y')
            # interleaved so x' = t0[:, 0::2], y' = t0[:, 1::2].
            # One consolidated tile: all 6 writers are DVE -> one sem.
            xs = t0[:, 0:512:2]
            ys = t0[:, 1:512:2]
            planes = cpool.tile([128, 1536], F32, tag="planes")
            nc.vector.memset(planes[:, 0:256], 1.0)
            nc.vector.tensor_copy(planes[:, 256:512], xs)
            nc.vector.tensor_copy(planes[:, 512:768], ys)
            nc.vector.tensor_mul(planes[:, 768:1024], xs, xs)
            nc.vector.tensor_mul(planes[:, 1024:1280], xs, ys)
            nc.vector.tensor_mul(planes[:, 1280:1536], ys, ys)

            # ---- F matrix [128, 32768]: rows 32c+k = feature k ----------
            # Bounce via DRAM so each fmat row-group has exactly ONE writer
            # DMA (matmul sync-wait slots are scarce). DRAM layout is
            # feature-major [k=6][a=128][b=256] so each fmat row is ONE
            # contiguous 128KB run (6 descriptors per load, not 768).
            fbounce = dpool.tile([128, 1536], F32, tag="fb")
            fb_flat = fbounce[:].rearrange("a b -> (a b)")
            nc.sync.dma_start(
                fb_flat.rearrange("(k a b) -> a k b", k=6, b=256), planes[:]
            )
            fmat = cpool.tile([128, PX_CORE], MM1_DT, tag="fmat")
            for c in range(NCHUNK):
                base = 32 * c
                dst = fmat[base : base + 6, :]
                src = fb_flat.rearrange("(k ab) -> k ab", k=6)
                nc.sync.dma_start(dst, src)

            # ---- persistent accumulators --------------------------------
            maxbuf = cpool.tile([128, 2048], BF16, tag="maxb")
            nc.vector.memset(maxbuf[:], 0.0)
            # channel-major image staging in DRAM: row 32j+c holds channel c
            # of tiles t%4==j; col 512u+i = pixel i of tile 4u+j.
            imgdram = dpool.tile([128, 8192], F32, tag="imgd")

            # ---- main loop ----------------------------------------------
            for t in range(NT):
                # MM1: r = G . F, row-tiled over 4 gaussian chunks.
                r0 = rpool.tile([128, 1024], F32, tag="r")
                r1 = rpool.tile([128, 1024], F32, tag="r")
                rhalves = (r0, r1)
                for c in range(NCHUNK):
                    rh = rhalves[c // 2]
                    off = 512 * (c % 2)
                    nc.tensor.matmul(
                        rh[:, off : off + 512],
                        lhsT=g_sb[32 * c : 32 * c + 6, :],
                        rhs=fmat[32 * c : 32 * c + 6, PT * t : PT * (t + 1)],
                        start=True,
                        stop=True,
                        tile_position=(32 * c, 0),
                    )

                # exp on ACT: PSUM fp32 -> SBUF bf16
                prob = ppool.tile([128, 2048], BF16, tag="prob")
                nc.scalar.activation(
                    prob[:, 0:1024], r0[:], mybir.ActivationFunctionType.Exp
                )
                nc.scalar.activation(
                    prob[:, 1024:2048], r1[:], mybir.ActivationFunctionType.Exp
                )

                # running max (DVE, bf16 2x mode)
                nc.vector.tensor_tensor(
                    maxbuf[:], maxbuf[:], prob[:], op=mybir.AluOpType.max
                )

                # MM2: img[3, 512] at partition base 32*(t%4), 4 tiles/bank
                j = t % 4
                u = t // 4
                if j == 0:
                    imgp = ipool.tile([128, 512], F32, tag="img")
                for c in range(NCHUNK):
                    # M padded to 32 (cols 3..31 of wts are zero) so the
                    # whole 32-partition group is written -> defined PSUM.
                    nc.tensor.matmul(
                        imgp[32 * j : 32 * j + 32, :],
                        lhsT=w_sb[:, 32 * c : 32 * c + 32],
                        rhs=prob[:, 512 * c : 512 * (c + 1)],
                        start=(c == 0),
                        stop=(c == 3),
                        tile_position=(0, 32 * j),
                    )
                if j == 3:
                    # evacuate 4 image tiles with one [128, 512] DVE copy,
                    # then one contiguous-run DMA to the DRAM staging buffer
                    ev = epool.tile([128, 512], F32, tag="ev")
                    nc.vector.tensor_copy(ev[:], imgp[:])
                    nc.sync.dma_start(imgdram[:, 512 * u : 512 * (u + 1)], ev[:])

            # ---- global max ---------------------------------------------
            # per-partition max -> DRAM roundtrip to gather onto one
            # partition -> free-dim reduce -> AllReduce(max) across cores.
            maxcol = cpool.tile([128, 1], F32, tag="maxcol")
            nc.vector.tensor_reduce(
                maxcol[:],
                maxbuf[:],
                axis=mybir.AxisListType.X,
                op=mybir.AluOpType.max,
            )
            d1 = dpool.tile([1, 128], F32, tag="d1")
            nc.sync.dma_start(
                d1[:].rearrange("a b -> (a b)"),
                maxcol[:].rearrange("a b -> (a b)"),
            )
            row = cpool.tile([1, 128], F32, tag="row")
            nc.sync.dma_start(row[:], d1[:])
            pl = cpool.tile([1, 1], F32, tag="pl")
            nc.vector.tensor_reduce(
                pl[:], row[:], axis=mybir.AxisListType.X, op=mybir.AluOpType.max
            )
            # replicate local max along free dim for the collective buffer
            ones_row = cpool.tile([1, 128], F32, tag="onesrow")
            nc.vector.memset(ones_row[:], 1.0)
            lrow = cpool.tile([1, 128], F32, tag="lrow")
            nc.vector.tensor_scalar_mul(lrow[:], ones_row[:], pl[:])
            cc_in = dpool.tile([1, 128], F32, tag="ccin")
            cc_out = dpool.tile([1, 128], F32, tag="ccout")
            nc.sync.dma_start(cc_in[:], lrow[:])
            nc.gpsimd.collective_compute(
                "AllReduce",
                mybir.AluOpType.max,
                replica_groups=[list(range(N_CORES))],
                ins=[cc_in[:].opt()],
                outs=[cc_out[:].opt()],
            )
            pmax = cpool.tile([1, 1], F32, tag="pmax")
            nc.sync.dma_start(pmax[0:1, 0:1], cc_out[0:1, 0:1])

            # s = 1 / (2 * pmax), broadcast to all 128 partitions via DRAM
            srec = cpool.tile([1, 1], F32, tag="srec")
            nc.vector.tensor_scalar_mul(srec[:], pmax[:], 2.0)
            nc.vector.reciprocal(srec[:], srec[:])
            srow = cpool.tile([1, 128], F32, tag="srow")
            nc.vector.tensor_scalar_mul(srow[:], ones_row[:], srec[:])
            d2 = dpool.tile([1, 128], F32, tag="d2")
            nc.sync.dma_start(d2[:], srow[:])
            sbc = cpool.tile([128, 1], F32, tag="sbc")
            nc.sync.dma_start(
                sbc[:].rearrange("a b -> (a b)"),
                d2[:].rearrange("a b -> (a b)"),
            )

            # ---- compact the 12 real rows of each slab to [96, 1024] ----
            # partition 12g+3j+c, col 512h+i  <-  imgdram[32j+c, 1024g+512h+i]
            fin = cpool.tile([96, 1024], F32, tag="fin")
            for g in range(8):
                for j in range(4):
                    nc.sync.dma_start(
                        fin[12 * g + 3 * j : 12 * g + 3 * j + 3, :],
                        imgdram[32 * j : 32 * j + 3, 1024 * g : 1024 * (g + 1)],
                    )

            # ---- final pass: sigmoid(acc/pmax) = 0.5 + 0.5*tanh(acc*s) --
            nc.vector.tensor_scalar_mul(fin[:], fin[:], sbc[0:96, :])
            nc.scalar.activation(
                fin[:], fin[:], mybir.ActivationFunctionType.Tanh
            )
            nc.vector.tensor_scalar(
                fin[:],
                fin[:],
                0.5,
                0.5,
                op0=mybir.AluOpType.mult,
                op1=mybir.AluOpType.add,
            )
            nc.sync.dma_start(out[:], fin[:])

    _legalize_waits(nc)
    return nc


# walrus encodes sync waits into fixed ISA struct slots (fused matmuls /
# TT hold only ONE). Hoist excess waits onto same-engine NOPs spliced
# immediately before the instruction — semantically identical (the engine
# stalls at the NOP instead).
def _legalize_waits(nc, cap=1):
    for blk in nc.main_func.blocks:
        insts = blk.instructions
        out = []
        for ins in insts:
            si = ins.sync_info
            if si is not None and len(si.on_wait) > cap:
                waits = list(si.on_wait)
                excess, keep = waits[:-cap], waits[-cap:]
                for w in excess:
                    eng = nc.engines[ins.engine]
                    n = eng.nop(hint="wait_legalize")
                    tail = nc.main_func.blocks[-1].instructions
                    assert tail[-1] is n.ins
                    tail.pop()
                    n.ins.sync_info = mybir.SyncInfo(
                        on_wait=[w], on_update=[]
                    )
                    out.append(n.ins)
                si.on_wait = keep
            out.append(ins)
        insts[:] = out


def _host_prep(mean, alpha, scale, theta, rgb, pixels):
    """Fold gaussian params into matmul coefficients (float64 on host)."""
    mean = np.asarray(mean, np.float64)
    alpha = np.asarray(alpha, np.float64)
    scale = np.asarray(scale, np.float64)
    theta = np.asarray(theta, np.float64)
    rgb = np.asarray(rgb, np.float64)
    pixels = np.asarray(pixels, np.float32)

    two_pi = 2.0 * np.pi
    ta = two_pi * theta[:, 0]
    c, s = np.cos(ta), np.sin(ta)
    sx2 = scale[:, 0] ** 2
    sy2 = scale[:, 1] ** 2
    A = c * c * sx2 + s * s * sy2
    Bc = c * s * (sx2 - sy2)
    D = s * s * sx2 + c * c * sy2
    det = A * D - Bc * Bc
    i00 = D / det
    iBs = -2.0 * Bc / det          # inv01 + inv10
    i11 = A / det
    lognorm = -np.log(two_pi) - 0.5 * np.log(det)

    px0 = mean[:, 0, 0] - 0.5      # gaussian means in centered coords
    py0 = mean[:, 1, 0] - 0.5
    c_1 = (
        -0.5 * (i00 * px0 * px0 + iBs * px0 * py0 + i11 * py0 * py0) + lognorm
    )
    c_x = i00 * px0 + 0.5 * iBs * py0
    c_y = 0.5 * iBs * px0 + i11 * py0
    c_xx = -0.5 * i00
    c_xy = -0.5 * iBs
    c_yy = -0.5 * i11

    gco = np.zeros((128, 128), np.float32)
    coefs = np.stack([c_1, c_x, c_y, c_xx, c_xy, c_yy])  # [6, 512]
    for ch in range(NCHUNK):
        gco[32 * ch : 32 * ch + 6, :] = coefs[:, 128 * ch : 128 * (ch + 1)]

    w = (rgb * alpha).astype(ml_dtypes.bfloat16)          # [512, 3]
    wts = np.zeros((128, 128), ml_dtypes.bfloat16)
    for ch in range(NCHUNK):
        wts[:, 32 * ch : 32 * ch + 3] = w[128 * ch : 128 * (ch + 1), :]

    # centered pixels, per-core slices in [128, 512] partition layout
    pc = (pixels.astype(np.float32) - np.float32(0.5)).reshape(-1)  # (H*W*2,)
    pix_cores = pc.reshape(N_CORES, 128, 512)
    return gco, wts, pix_cores


def make_in_maps(mean, alpha, scale, theta, rgb, pixels):
    gco, wts, pix_cores = _host_prep(mean, alpha, scale, theta, rgb, pixels)
    return [
        {"pix": np.ascontiguousarray(pix_cores[i]), "gco": gco, "wts": wts}
        for i in range(N_CORES)
    ]


_Q = np.arange(96)
_I = np.arange(1024)
_C = _Q % 3
_PX = (
    512 * (4 * (2 * (_Q // 12)[:, None] + _I[None, :] // 512)
           + ((_Q % 12) // 3)[:, None])
    + (_I[None, :] % 512)
)


def assemble(results):
    out = np.empty((N_CORES, PX_CORE, 3), np.float32)
    for n, r in enumerate(results):
        buf = np.asarray(r["out"], np.float32)
        out[n, _PX, np.broadcast_to(_C[:, None], _PX.shape)] = buf
    return out.reshape(H, W, 3)


def kernel(mean, alpha, scale, theta, rgb, pixels):
    global LAST_EXEC_NS, LAST_RESULTS
    in_maps = make_in_maps(mean, alpha, scale, theta, rgb, pixels)
    nc = _build_nc()
    trace = os.environ.get("KERNEL_TRACE", "0") == "1"
    res = run_bass_kernel_spmd(nc, in_maps, list(range(N_CORES)), trace=trace)
    LAST_EXEC_NS = res.exec_time_ns
    LAST_RESULTS = res
    return assemble(res.results)


# revision 35
# speedup vs baseline: 3.6583x; 1.4570x over previous
"""GaussianImage splat kernel for 8 Trainium2 NeuronCores.

Math (matches reference.py):
    cov      = R(theta) S S^T R(theta)^T per gaussian
    q(n, p)  = (p - mu_n)^T cov_n^{-1} (p - mu_n)
    prob     = exp(-q/2) / (2*pi*sqrt(det))
    splat    = prob / max(prob)           (global max over all n, p)
    img      = einsum('nhw,nc->hwc', splat, rgb*alpha)
    out      = sigmoid(img)

Device formulation:
    r(n, p)  = G[n] . F[p]   with F = [1, x', y', x'^2, x'y', y'^2] (centered
               monomials, x' = x - 0.5) and G the folded per-gaussian
               coefficients including log norm -> prob = exp(r).
    MM1 (PE, row-tiled 4x128-gaussian chunks), exp on ACT (PSUM->SBUF bf16),
    running elementwise max on DVE, MM2 (PE) contracts gaussians against
    w = rgb*alpha, 4 image tiles packed per PSUM bank at partition bases
    {0,32,64,96}, DVE evacuates, DMA repacks to pixel-major, AllReduce(max)
    over the 8 cores, then sigmoid via tanh (same ACT table set as exp).

Sharding: pixels split across the 8 cores (each core: all 512 gaussians x
32768 pixels); only a 512-byte all-reduce(max) collective is needed.
"""

import os
import sys
import numpy as np

for _p in ("/opt/trn_rl_repo", "/root/.axon_site/_ro/trn_rl_repo"):
    if os.path.isdir(_p) and _p not in sys.path:
        sys.path.insert(0, _p)

import ml_dtypes  # noqa: E402

def _ensure_ntff_hook():
    """antenv.axon_hooks is missing in this image; shim it and register the
    ctypes NTFF profiler from trn_agent_boot so trace=True works."""
    import types

    try:
        import antenv.axon_hooks  # noqa: F401
        return
    except ImportError:
        pass
    import antenv

    mod = types.ModuleType("antenv.axon_hooks")
    mod._hook = None

    def set_axon_ntff_profile_hook(h):
        mod._hook = h

    def get_axon_ntff_profile_hook():
        return mod._hook

    mod.set_axon_ntff_profile_hook = set_axon_ntff_profile_hook
    mod.get_axon_ntff_profile_hook = get_axon_ntff_profile_hook
    sys.modules["antenv.axon_hooks"] = mod
    antenv.axon_hooks = mod
    try:
        from trn_agent_boot.trn_boot import _ntff_profile_via_ctypes

        set_axon_ntff_profile_hook(
            _ntff_profile_via_ctypes("/opt/axon/libaxon_pjrt.so")
        )
    except Exception:
        pass


_ensure_ntff_hook()

import concourse.bass as bass  # noqa: E402
import concourse.tile as tile  # noqa: E402
from concourse import bass_isa, library_config, mybir  # noqa: E402
from concourse.tile import add_dep_helper  # noqa: E402
from concourse.bass_utils import run_bass_kernel_spmd  # noqa: E402

F32 = mybir.dt.float32
BF16 = mybir.dt.bfloat16

N_GAUSS = 512
H = 512
W = 512
N_CORES = 8
PX_CORE = H * W // N_CORES      # 32768 pixels per core
PT = 512                        # pixels per tile
NT = PX_CORE // PT              # 64 tiles per core
NCHUNK = 4                      # gaussian chunks of 128

# MM1 moving/stationary dtype: F32 is exact (4 cyc/row), float32r is fast
# (1 cyc/row) with relaxed precision. Toggle via env for experiments.
MM1_DT = {"f32": F32, "f32r": mybir.dt.float32r}[os.environ.get("MM1_DT", "f32")]

LAST_EXEC_NS = None
LAST_RESULTS = None


def _build_nc():
    nc = bass.Bass(num_devices=N_CORES)

    # Per-core inputs (host pre-sliced / pre-massaged).
    pix = nc.declare_dram_parameter("pix", [128, 512], F32, isOutput=False)
    gco = nc.declare_dram_parameter("gco", [128, 128], MM1_DT, isOutput=False)
    wts = nc.declare_dram_parameter("wts", [128, 128], BF16, isOutput=False)
    out = nc.declare_dram_parameter("out", [96, 1024], F32, isOutput=True)

    with tile.TileContext(nc) as tc:
        with (
            tc.tile_pool(name="const", bufs=1) as cpool,
            tc.tile_pool(name="rps", bufs=3, space="PSUM") as rpool,
            tc.tile_pool(name="ips", bufs=2, space="PSUM") as ipool,
            tc.tile_pool(name="prob", bufs=3) as ppool,
            tc.tile_pool(name="evac", bufs=3) as epool,
            tc.tile_pool(name="dram", bufs=1, space="DRAM") as dpool,
        ):
            # ---- load inputs --------------------------------------------
            t0 = cpool.tile([128, 512], F32, tag="t0")       # centered pixels
            nc.sync.dma_start(t0[:], pix[:])
            g_sb = cpool.tile([128, 128], MM1_DT, tag="g")
            nc.sync.dma_start(g_sb[:], gco[:])
            w_sb = cpool.tile([128, 128], BF16, tag="w")
            nc.sync.dma_start(w_sb[:], wts[:])

            # ---- feature planes [128, 6*256] ----------------------------
            # partition p holds pixels 256p..256p+255; t0 is (x', y')
            # interleaved so x' = t0[:, 0::2], y' = t0[:, 1::2].
            # One consolidated tile: all 6 writers are DVE -> one sem.
            xs = t0[:, 0:512:2]
            ys = t0[:, 1:512:2]
            planes = cpool.tile([128, 1536], F32, tag="planes")
            nc.vector.memset(planes[:, 0:256], 1.0)
            nc.vector.tensor_copy(planes[:, 256:512], xs)
            nc.vector.tensor_copy(planes[:, 512:768], ys)
            nc.vector.tensor_mul(planes[:, 768:1024], xs, xs)
            nc.vector.tensor_mul(planes[:, 1024:1280], xs, ys)
            nc.vector.tensor_mul(planes[:, 1280:1536], ys, ys)

            # ---- F matrix [128, 32768]: rows 32c+k = feature k ----------
            # Bounce via DRAM so each fmat row-group has exactly ONE writer
            # DMA (matmul sync-wait slots are scarce). DRAM layout is
            # feature-major [k=6][a=128][b=256] so each fmat row is ONE
            # contiguous 128KB run (6 descriptors per load, not 768).
            fbounce = dpool.tile([128, 1536], F32, tag="fb")
            fb_flat = fbounce[:].rearrange("a b -> (a b)")
            nc.sync.dma_start(
                fb_flat.rearrange("(k a b) -> a k b", k=6, b=256), planes[:]
            )
            fmat = cpool.tile([128, PX_CORE], MM1_DT, tag="fmat")
            for c in range(NCHUNK):
                base = 32 * c
                dst = fmat[base : base + 6, :].bitcast(F32)
                src = fb_flat.rearrange("(k ab) -> k ab", k=6)
                nc.sync.dma_start(dst, src)

            # ---- persistent accumulators --------------------------------
            maxbuf = cpool.tile([128, 2048], BF16, tag="maxb")
            nc.vector.memset(maxbuf[:], 0.0)
            # channel-major image staging in DRAM: row 32j+c holds channel c
            # of tiles t%4==j; col 512u+i = pixel i of tile 4u+j.
            imgdram = dpool.tile([128, 8192], F32, tag="imgd")

            # ---- main loop (software-pipelined: consume prob one
            # tile late so PE's next MM1 isn't queued behind MM2) ---------
            probs = [None] * NT

            def consume(t):
                """MM2 + running max + evacuation for tile t."""
                prob = probs[t]
                j = t % 4
                u = t // 4
                nc.vector.tensor_tensor(
                    maxbuf[:], maxbuf[:], prob[:], op=mybir.AluOpType.max
                )
                if j == 0:
                    consume.imgp = ipool.tile([128, 512], F32, tag="img")
                imgp = consume.imgp
                for c in range(NCHUNK):
                    # M padded to 32 (cols 3..31 of wts are zero) so the
                    # whole 32-partition group is written -> defined PSUM.
                    nc.tensor.matmul(
                        imgp[32 * j : 32 * j + 32, :],
                        lhsT=w_sb[:, 32 * c : 32 * c + 32],
                        rhs=prob[:, 512 * c : 512 * (c + 1)],
                        start=(c == 0),
                        stop=(c == 3),
                        tile_position=(0, 32 * j),
                    )
                if j == 3:
                    ev = epool.tile([128, 512], F32, tag="ev")
                    nc.vector.tensor_copy(ev[:], imgp[:])
                    nc.sync.dma_start(imgdram[:, 512 * u : 512 * (u + 1)], ev[:])

            for t in range(NT):
                # MM1: r = G . F, row-tiled over 4 gaussian chunks.
                r0 = rpool.tile([128, 1024], F32, tag="r")
                r1 = rpool.tile([128, 1024], F32, tag="r")
                rhalves = (r0, r1)
                for c in range(NCHUNK):
                    rh = rhalves[c // 2]
                    off = 512 * (c % 2)
                    nc.tensor.matmul(
                        rh[:, off : off + 512],
                        lhsT=g_sb[32 * c : 32 * c + 6, :],
                        rhs=fmat[32 * c : 32 * c + 6, PT * t : PT * (t + 1)],
                        start=True,
                        stop=True,
                        tile_position=(32 * c, 0),
                    )

                # exp on ACT: PSUM fp32 -> SBUF bf16
                prob = ppool.tile([128, 2048], BF16, tag="prob")
                nc.scalar.activation(
                    prob[:, 0:1024], r0[:], mybir.ActivationFunctionType.Exp
                )
                nc.scalar.activation(
                    prob[:, 1024:2048], r1[:], mybir.ActivationFunctionType.Exp
                )
                probs[t] = prob

                if t >= 1:
                    consume(t - 1)
            consume(NT - 1)

            # ---- global max ---------------------------------------------
            # per-partition max -> straight into the collective buffer; the
            # AllReduce(max) reduces across cores elementwise, the scalar
            # reduce of the 128-vector happens after.
            maxcol = cpool.tile([128, 1], F32, tag="maxcol")
            nc.vector.tensor_reduce(
                maxcol[:],
                maxbuf[:],
                axis=mybir.AxisListType.X,
                op=mybir.AluOpType.max,
            )
            cc_in = dpool.tile([1, 128], F32, tag="ccin")
            cc_out = dpool.tile([1, 128], F32, tag="ccout")
            nc.sync.dma_start(
                cc_in[:].rearrange("a b -> (a b)"),
                maxcol[:].rearrange("a b -> (a b)"),
            )
            nc.gpsimd.collective_compute(
                "AllReduce",
                mybir.AluOpType.max,
                replica_groups=[list(range(N_CORES))],
                ins=[cc_in[:].opt()],
                outs=[cc_out[:].opt()],
            )
            row = cpool.tile([1, 128], F32, tag="row")
            nc.sync.dma_start(row[:], cc_out[:])
            ones_row = cpool.tile([1, 128], F32, tag="onesrow")
            nc.vector.memset(ones_row[:], 1.0)
            pmax = cpool.tile([1, 1], F32, tag="pmax")
            nc.vector.tensor_reduce(
                pmax[:], row[:], axis=mybir.AxisListType.X, op=mybir.AluOpType.max
            )

            # s = 1 / (2 * pmax), broadcast to all 128 partitions via DRAM
            srec = cpool.tile([1, 1], F32, tag="srec")
            nc.vector.tensor_scalar_mul(srec[:], pmax[:], 2.0)
            nc.vector.reciprocal(srec[:], srec[:])
            srow = cpool.tile([1, 128], F32, tag="srow")
            nc.vector.tensor_scalar_mul(srow[:], ones_row[:], srec[:])
            d2 = dpool.tile([1, 128], F32, tag="d2")
            nc.sync.dma_start(d2[:], srow[:])
            sbc = cpool.tile([128, 1], F32, tag="sbc")
            nc.sync.dma_start(
                sbc[:].rearrange("a b -> (a b)"),
                d2[:].rearrange("a b -> (a b)"),
            )

            # ---- compact the 12 real rows of each slab to [96, 1024] ----
            # partition 12g+3j+c, col 512h+i  <-  imgdram[32j+c, 1024g+512h+i]
            fin = cpool.tile([96, 1024], F32, tag="fin")
            for g in range(8):
                for j in range(4):
                    nc.sync.dma_start(
                        fin[12 * g + 3 * j : 12 * g + 3 * j + 3, :],
                        imgdram[32 * j : 32 * j + 3, 1024 * g : 1024 * (g + 1)],
                    )

            # ---- final pass: sigmoid(acc/pmax) = 0.5 + 0.5*tanh(acc*s) --
            nc.vector.tensor_scalar_mul(fin[:], fin[:], sbc[0:96, :])
            nc.scalar.activation(
                fin[:], fin[:], mybir.ActivationFunctionType.Tanh
            )
            nc.vector.tensor_scalar(
                fin[:],
                fin[:],
                0.5,
                0.5,
                op0=mybir.AluOpType.mult,
                op1=mybir.AluOpType.add,
            )
            nc.sync.dma_start(out[:], fin[:])

    _legalize_waits(nc)
    return nc


# walrus encodes sync waits into fixed ISA struct slots (fused matmuls /
# TT hold only ONE). Hoist excess waits onto same-engine NOPs spliced
# immediately before the instruction — semantically identical (the engine
# stalls at the NOP instead).
def _legalize_waits(nc, cap=1):
    for blk in nc.main_func.blocks:
        insts = blk.instructions
        out = []
        for ins in insts:
            si = ins.sync_info
            if si is not None and len(si.on_wait) > cap:
                waits = list(si.on_wait)
                excess, keep = waits[:-cap], waits[-cap:]
                for w in excess:
                    eng = nc.engines[ins.engine]
                    n = eng.nop(hint="wait_legalize")
                    tail = nc.main_func.blocks[-1].instructions
                    assert tail[-1] is n.ins
                    tail.pop()
                    n.ins.sync_info = mybir.SyncInfo(
                        on_wait=[w], on_update=[]
                    )
                    out.append(n.ins)
                si.on_wait = keep
            out.append(ins)
        insts[:] = out


def _host_prep(mean, alpha, scale, theta, rgb, pixels):
    """Fold gaussian params into matmul coefficients (float64 on host)."""
    mean = np.asarray(mean, np.float64)
    alpha = np.asarray(alpha, np.float64)
    scale = np.asarray(scale, np.float64)
    theta = np.asarray(theta, np.float64)
    rgb = np.asarray(rgb, np.float64)
    pixels = np.asarray(pixels, np.float32)

    two_pi = 2.0 * np.pi
    ta = two_pi * theta[:, 0]
    c, s = np.cos(ta), np.sin(ta)
    sx2 = scale[:, 0] ** 2
    sy2 = scale[:, 1] ** 2
    A = c * c * sx2 + s * s * sy2
    Bc = c * s * (sx2 - sy2)
    D = s * s * sx2 + c * c * sy2
    det = A * D - Bc * Bc
    i00 = D / det
    iBs = -2.0 * Bc / det          # inv01 + inv10
    i11 = A / det
    lognorm = -np.log(two_pi) - 0.5 * np.log(det)

    px0 = mean[:, 0, 0] - 0.5      # gaussian means in centered coords
    py0 = mean[:, 1, 0] - 0.5
    c_1 = (
        -0.5 * (i00 * px0 * px0 + iBs * px0 * py0 + i11 * py0 * py0) + lognorm
    )
    c_x = i00 * px0 + 0.5 * iBs * py0
    c_y = 0.5 * iBs * px0 + i11 * py0
    c_xx = -0.5 * i00
    c_xy = -0.5 * iBs
    c_yy = -0.5 * i11

    gco = np.zeros((128, 128), np.float32)
    coefs = np.stack([c_1, c_x, c_y, c_xx, c_xy, c_yy])  # [6, 512]
    for ch in range(NCHUNK):
        gco[32 * ch : 32 * ch + 6, :] = coefs[:, 128 * ch : 128 * (ch + 1)]

    w = (rgb * alpha).astype(ml_dtypes.bfloat16)          # [512, 3]
    wts = np.zeros((128, 128), ml_dtypes.bfloat16)
    for ch in range(NCHUNK):
        wts[:, 32 * ch : 32 * ch + 3] = w[128 * ch : 128 * (ch + 1), :]

    # centered pixels, per-core slices in [128, 512] partition layout
    pc = (pixels.astype(np.float32) - np.float32(0.5)).reshape(-1)  # (H*W*2,)
    pix_cores = pc.reshape(N_CORES, 128, 512)
    return gco, wts, pix_cores


def make_in_maps(mean, alpha, scale, theta, rgb, pixels):
    gco, wts, pix_cores = _host_prep(mean, alpha, scale, theta, rgb, pixels)
    return [
        {"pix": np.ascontiguousarray(pix_cores[i]), "gco": gco, "wts": wts}
        for i in range(N_CORES)
    ]


_Q = np.arange(96)
_I = np.arange(1024)
_C = _Q % 3
_PX = (
    512 * (4 * (2 * (_Q // 12)[:, None] + _I[None, :] // 512)
           + ((_Q % 12) // 3)[:, None])
    + (_I[None, :] % 512)
)


def assemble(results):
    out = np.empty((N_CORES, PX_CORE, 3), np.float32)
    for n, r in enumerate(results):
        buf = np.asarray(r["out"], np.float32)
        out[n, _PX, np.broadcast_to(_C[:, None], _PX.shape)] = buf
    return out.reshape(H, W, 3)


def kernel(mean, alpha, scale, theta, rgb, pixels):
    global LAST_EXEC_NS, LAST_RESULTS
    in_maps = make_in_maps(mean, alpha, scale, theta, rgb, pixels)
    nc = _build_nc()
    trace = os.environ.get("KERNEL_TRACE", "0") == "1"
    res = run_bass_kernel_spmd(nc, in_maps, list(range(N_CORES)), trace=trace)
    LAST_EXEC_NS = res.exec_time_ns
    LAST_RESULTS = res
    return assemble(res.results)


# revision 36
# speedup vs baseline: 3.7481x; 1.0245x over previous
"""GaussianImage splat kernel for 8 Trainium2 NeuronCores.

Math (matches reference.py):
    cov      = R(theta) S S^T R(theta)^T per gaussian
    q(n, p)  = (p - mu_n)^T cov_n^{-1} (p - mu_n)
    prob     = exp(-q/2) / (2*pi*sqrt(det))
    splat    = prob / max(prob)           (global max over all n, p)
    img      = einsum('nhw,nc->hwc', splat, rgb*alpha)
    out      = sigmoid(img)

Device formulation:
    r(n, p)  = G[n] . F[p]   with F = [1, x', y', x'^2, x'y', y'^2] (centered
               monomials, x' = x - 0.5) and G the folded per-gaussian
               coefficients including log norm -> prob = exp(r).
    MM1 (PE, row-tiled 4x128-gaussian chunks), exp on ACT (PSUM->SBUF bf16),
    running elementwise max on DVE, MM2 (PE) contracts gaussians against
    w = rgb*alpha, 4 image tiles packed per PSUM bank at partition bases
    {0,32,64,96}, DVE evacuates, DMA repacks to pixel-major, AllReduce(max)
    over the 8 cores, then sigmoid via tanh (same ACT table set as exp).

Sharding: pixels split across the 8 cores (each core: all 512 gaussians x
32768 pixels); only a 512-byte all-reduce(max) collective is needed.
"""

import os
import sys
import numpy as np

for _p in ("/opt/trn_rl_repo", "/root/.axon_site/_ro/trn_rl_repo"):
    if os.path.isdir(_p) and _p not in sys.path:
        sys.path.insert(0, _p)

import ml_dtypes  # noqa: E402

def _ensure_ntff_hook():
    """antenv.axon_hooks is missing in this image; shim it and register the
    ctypes NTFF profiler from trn_agent_boot so trace=True works."""
    import types

    try:
        import antenv.axon_hooks  # noqa: F401
        return
    except ImportError:
        pass
    import antenv

    mod = types.ModuleType("antenv.axon_hooks")
    mod._hook = None

    def set_axon_ntff_profile_hook(h):
        mod._hook = h

    def get_axon_ntff_profile_hook():
        return mod._hook

    mod.set_axon_ntff_profile_hook = set_axon_ntff_profile_hook
    mod.get_axon_ntff_profile_hook = get_axon_ntff_profile_hook
    sys.modules["antenv.axon_hooks"] = mod
    antenv.axon_hooks = mod
    try:
        from trn_agent_boot.trn_boot import _ntff_profile_via_ctypes

        set_axon_ntff_profile_hook(
            _ntff_profile_via_ctypes("/opt/axon/libaxon_pjrt.so")
        )
    except Exception:
        pass


_ensure_ntff_hook()

import concourse.bass as bass  # noqa: E402
import concourse.tile as tile  # noqa: E402
from concourse import bass_isa, library_config, mybir  # noqa: E402
from concourse.tile import add_dep_helper  # noqa: E402
from concourse.bass_utils import run_bass_kernel_spmd  # noqa: E402

F32 = mybir.dt.float32
BF16 = mybir.dt.bfloat16

N_GAUSS = 512
H = 512
W = 512
N_CORES = 8
PX_CORE = H * W // N_CORES      # 32768 pixels per core
PT = 512                        # pixels per tile
NT = PX_CORE // PT              # 64 tiles per core
NCHUNK = 4                      # gaussian chunks of 128

# MM1 moving/stationary dtype: F32 is exact (4 cyc/row), float32r is fast
# (1 cyc/row) with relaxed precision. Toggle via env for experiments.
MM1_DT = {"f32": F32, "f32r": mybir.dt.float32r}[os.environ.get("MM1_DT", "f32")]

LAST_EXEC_NS = None
LAST_RESULTS = None


def _build_nc():
    nc = bass.Bass(num_devices=N_CORES)

    # Per-core inputs (host pre-sliced / pre-massaged).
    pix = nc.declare_dram_parameter("pix", [128, 512], F32, isOutput=False)
    gco = nc.declare_dram_parameter("gco", [128, 128], MM1_DT, isOutput=False)
    wts = nc.declare_dram_parameter("wts", [128, 128], BF16, isOutput=False)
    out = nc.declare_dram_parameter("out", [96, 1024], F32, isOutput=True)

    with tile.TileContext(nc) as tc:
        with (
            tc.tile_pool(name="const", bufs=1) as cpool,
            tc.tile_pool(name="rps", bufs=3, space="PSUM") as rpool,
            tc.tile_pool(name="ips", bufs=2, space="PSUM") as ipool,
            tc.tile_pool(name="prob", bufs=3) as ppool,
            tc.tile_pool(name="evac", bufs=3) as epool,
            tc.tile_pool(name="dram", bufs=1, space="DRAM") as dpool,
        ):
            # ---- load inputs --------------------------------------------
            t0 = cpool.tile([128, 512], F32, tag="t0")       # centered pixels
            nc.sync.dma_start(t0[:], pix[:])
            g_sb = cpool.tile([128, 128], MM1_DT, tag="g")
            nc.sync.dma_start(g_sb[:], gco[:])
            w_sb = cpool.tile([128, 128], BF16, tag="w")
            nc.sync.dma_start(w_sb[:], wts[:])

            # ---- feature planes [128, 6*256] ----------------------------
            # partition p holds pixels 256p..256p+255; t0 is (x', y')
            # interleaved so x' = t0[:, 0::2], y' = t0[:, 1::2].
            # One consolidated tile: all 6 writers are DVE -> one sem.
            xs = t0[:, 0:512:2]
            ys = t0[:, 1:512:2]
            planes = cpool.tile([128, 1536], F32, tag="planes")
            nc.vector.memset(planes[:, 0:256], 1.0)
            nc.vector.tensor_copy(planes[:, 256:512], xs)
            nc.vector.tensor_copy(planes[:, 512:768], ys)
            nc.vector.tensor_mul(planes[:, 768:1024], xs, xs)
            nc.vector.tensor_mul(planes[:, 1024:1280], xs, ys)
            nc.vector.tensor_mul(planes[:, 1280:1536], ys, ys)

            # ---- F matrix [128, 32768]: rows 32c+k = feature k ----------
            # Bounce via DRAM so each fmat row-group has exactly ONE writer
            # DMA (matmul sync-wait slots are scarce). DRAM layout is
            # feature-major [k=6][a=128][b=256] so each fmat row is ONE
            # contiguous 128KB run (6 descriptors per load, not 768).
            fbounce = dpool.tile([128, 1536], F32, tag="fb")
            fb_flat = fbounce[:].rearrange("a b -> (a b)")
            nc.sync.dma_start(
                fb_flat.rearrange("(k a b) -> a k b", k=6, b=256), planes[:]
            )
            # split each row-group load into col-halves so tile-0 matmuls
            # can start as soon as the first half lands
            fmat = cpool.tile([128, PX_CORE], MM1_DT, tag="fmat")
            HLF = PX_CORE // 2
            for half in range(2):
                for c in range(NCHUNK):
                    base = 32 * c
                    dst = fmat[
                        base : base + 6, HLF * half : HLF * (half + 1)
                    ].bitcast(F32)
                    src = fb_flat.rearrange(
                        "(k h ab) -> k h ab", k=6, h=2
                    )[:, half, :]
                    nc.sync.dma_start(dst, src)

            # ---- persistent accumulators --------------------------------
            maxbuf = cpool.tile([128, 2048], BF16, tag="maxb")
            nc.vector.memset(maxbuf[:], 0.0)
            # channel-major image staging in DRAM: row 32j+c holds channel c
            # of tiles t%4==j; col 512u+i = pixel i of tile 4u+j.
            imgdram = dpool.tile([128, 8192], F32, tag="imgd")

            # ---- main loop (software-pipelined: consume prob one
            # tile late so PE's next MM1 isn't queued behind MM2) ---------
            probs = [None] * NT

            def consume(t):
                """MM2 + running max + evacuation for tile t."""
                prob = probs[t]
                j = t % 4
                u = t // 4
                nc.vector.tensor_tensor(
                    maxbuf[:], maxbuf[:], prob[:], op=mybir.AluOpType.max
                )
                if j == 0:
                    consume.imgp = ipool.tile([128, 512], F32, tag="img")
                imgp = consume.imgp
                for c in range(NCHUNK):
                    # M padded to 32 (cols 3..31 of wts are zero) so the
                    # whole 32-partition group is written -> defined PSUM.
                    nc.tensor.matmul(
                        imgp[32 * j : 32 * j + 32, :],
                        lhsT=w_sb[:, 32 * c : 32 * c + 32],
                        rhs=prob[:, 512 * c : 512 * (c + 1)],
                        start=(c == 0),
                        stop=(c == 3),
                        tile_position=(0, 32 * j),
                    )
                if j == 3:
                    ev = epool.tile([128, 512], F32, tag="ev")
                    nc.vector.tensor_copy(ev[:], imgp[:])
                    nc.sync.dma_start(imgdram[:, 512 * u : 512 * (u + 1)], ev[:])

            for t in range(NT):
                # MM1: r = G . F, row-tiled over 4 gaussian chunks.
                r0 = rpool.tile([128, 1024], F32, tag="r")
                r1 = rpool.tile([128, 1024], F32, tag="r")
                rhalves = (r0, r1)
                for c in range(NCHUNK):
                    rh = rhalves[c // 2]
                    off = 512 * (c % 2)
                    nc.tensor.matmul(
                        rh[:, off : off + 512],
                        lhsT=g_sb[32 * c : 32 * c + 6, :],
                        rhs=fmat[32 * c : 32 * c + 6, PT * t : PT * (t + 1)],
                        start=True,
                        stop=True,
                        tile_position=(32 * c, 0),
                    )

                # exp on ACT: PSUM fp32 -> SBUF bf16
                prob = ppool.tile([128, 2048], BF16, tag="prob")
                nc.scalar.activation(
                    prob[:, 0:1024], r0[:], mybir.ActivationFunctionType.Exp
                )
                nc.scalar.activation(
                    prob[:, 1024:2048], r1[:], mybir.ActivationFunctionType.Exp
                )
                probs[t] = prob

                if t >= 1:
                    consume(t - 1)
            consume(NT - 1)

            # ---- global max ---------------------------------------------
            # per-partition max -> straight into the collective buffer; the
            # AllReduce(max) reduces across cores elementwise, the scalar
            # reduce of the 128-vector happens after.
            maxcol = cpool.tile([128, 1], F32, tag="maxcol")
            nc.vector.tensor_reduce(
                maxcol[:],
                maxbuf[:],
                axis=mybir.AxisListType.X,
                op=mybir.AluOpType.max,
            )
            cc_in = dpool.tile([1, 128], F32, tag="ccin")
            cc_out = dpool.tile([1, 128], F32, tag="ccout")
            nc.sync.dma_start(
                cc_in[:].rearrange("a b -> (a b)"),
                maxcol[:].rearrange("a b -> (a b)"),
            )
            nc.gpsimd.collective_compute(
                "AllReduce",
                mybir.AluOpType.max,
                replica_groups=[list(range(N_CORES))],
                ins=[cc_in[:].opt()],
                outs=[cc_out[:].opt()],
            )
            row = cpool.tile([1, 128], F32, tag="row")
            nc.sync.dma_start(row[:], cc_out[:])
            ones_row = cpool.tile([1, 128], F32, tag="onesrow")
            nc.vector.memset(ones_row[:], 1.0)
            pmax = cpool.tile([1, 1], F32, tag="pmax")
            nc.vector.tensor_reduce(
                pmax[:], row[:], axis=mybir.AxisListType.X, op=mybir.AluOpType.max
            )

            # s = 1 / (2 * pmax), broadcast to all 128 partitions via DRAM
            srec = cpool.tile([1, 1], F32, tag="srec")
            nc.vector.tensor_scalar_mul(srec[:], pmax[:], 2.0)
            nc.vector.reciprocal(srec[:], srec[:])
            srow = cpool.tile([1, 128], F32, tag="srow")
            nc.vector.tensor_scalar_mul(srow[:], ones_row[:], srec[:])
            d2 = dpool.tile([1, 128], F32, tag="d2")
            nc.sync.dma_start(d2[:], srow[:])
            sbc = cpool.tile([128, 1], F32, tag="sbc")
            nc.sync.dma_start(
                sbc[:].rearrange("a b -> (a b)"),
                d2[:].rearrange("a b -> (a b)"),
            )

            # ---- compact the 12 real rows of each slab to [96, 1024] ----
            # partition 12g+3j+c, col 512h+i  <-  imgdram[32j+c, 1024g+512h+i]
            fin = cpool.tile([96, 1024], F32, tag="fin")
            for g in range(8):
                for j in range(4):
                    nc.sync.dma_start(
                        fin[12 * g + 3 * j : 12 * g + 3 * j + 3, :],
                        imgdram[32 * j : 32 * j + 3, 1024 * g : 1024 * (g + 1)],
                    )

            # ---- final pass: sigmoid(acc/pmax) = 0.5 + 0.5*tanh(acc*s) --
            nc.vector.tensor_scalar_mul(fin[:], fin[:], sbc[0:96, :])
            nc.scalar.activation(
                fin[:], fin[:], mybir.ActivationFunctionType.Tanh
            )
            nc.vector.tensor_scalar(
                fin[:],
                fin[:],
                0.5,
                0.5,
                op0=mybir.AluOpType.mult,
                op1=mybir.AluOpType.add,
            )
            nc.sync.dma_start(out[:], fin[:])

    _legalize_waits(nc)
    return nc


# walrus encodes sync waits into fixed ISA struct slots (fused matmuls /
# TT hold only ONE). Hoist excess waits onto same-engine NOPs spliced
# immediately before the instruction — semantically identical (the engine
# stalls at the NOP instead).
def _legalize_waits(nc, cap=1):
    for blk in nc.main_func.blocks:
        insts = blk.instructions
        out = []
        for ins in insts:
            si = ins.sync_info
            if si is not None and len(si.on_wait) > cap:
                waits = list(si.on_wait)
                excess, keep = waits[:-cap], waits[-cap:]
                for w in excess:
                    eng = nc.engines[ins.engine]
                    n = eng.nop(hint="wait_legalize")
                    tail = nc.main_func.blocks[-1].instructions
                    assert tail[-1] is n.ins
                    tail.pop()
                    n.ins.sync_info = mybir.SyncInfo(
                        on_wait=[w], on_update=[]
                    )
                    out.append(n.ins)
                si.on_wait = keep
            out.append(ins)
        insts[:] = out


def _host_prep(mean, alpha, scale, theta, rgb, pixels):
    """Fold gaussian params into matmul coefficients (float64 on host)."""
    mean = np.asarray(mean, np.float64)
    alpha = np.asarray(alpha, np.float64)
    scale = np.asarray(scale, np.float64)
    theta = np.asarray(theta, np.float64)
    rgb = np.asarray(rgb, np.float64)
    pixels = np.asarray(pixels, np.float32)

    two_pi = 2.0 * np.pi
    ta = two_pi * theta[:, 0]
    c, s = np.cos(ta), np.sin(ta)
    sx2 = scale[:, 0] ** 2
    sy2 = scale[:, 1] ** 2
    A = c * c * sx2 + s * s * sy2
    Bc = c * s * (sx2 - sy2)
    D = s * s * sx2 + c * c * sy2
    det = A * D - Bc * Bc
    i00 = D / det
    iBs = -2.0 * Bc / det          # inv01 + inv10
    i11 = A / det
    lognorm = -np.log(two_pi) - 0.5 * np.log(det)

    px0 = mean[:, 0, 0] - 0.5      # gaussian means in centered coords
    py0 = mean[:, 1, 0] - 0.5
    c_1 = (
        -0.5 * (i00 * px0 * px0 + iBs * px0 * py0 + i11 * py0 * py0) + lognorm
    )
    c_x = i00 * px0 + 0.5 * iBs * py0
    c_y = 0.5 * iBs * px0 + i11 * py0
    c_xx = -0.5 * i00
    c_xy = -0.5 * iBs
    c_yy = -0.5 * i11

    gco = np.zeros((128, 128), np.float32)
    coefs = np.stack([c_1, c_x, c_y, c_xx, c_xy, c_yy])  # [6, 512]
    for ch in range(NCHUNK):
        gco[32 * ch : 32 * ch + 6, :] = coefs[:, 128 * ch : 128 * (ch + 1)]

    w = (rgb * alpha).astype(ml_dtypes.bfloat16)          # [512, 3]
    wts = np.zeros((128, 128), ml_dtypes.bfloat16)
    for ch in range(NCHUNK):
        wts[:, 32 * ch : 32 * ch + 3] = w[128 * ch : 128 * (ch + 1), :]

    # centered pixels, per-core slices in [128, 512] partition layout
    pc = (pixels.astype(np.float32) - np.float32(0.5)).reshape(-1)  # (H*W*2,)
    pix_cores = pc.reshape(N_CORES, 128, 512)
    return gco, wts, pix_cores


def make_in_maps(mean, alpha, scale, theta, rgb, pixels):
    gco, wts, pix_cores = _host_prep(mean, alpha, scale, theta, rgb, pixels)
    return [
        {"pix": np.ascontiguousarray(pix_cores[i]), "gco": gco, "wts": wts}
        for i in range(N_CORES)
    ]


_Q = np.arange(96)
_I = np.arange(1024)
_C = _Q % 3
_PX = (
    512 * (4 * (2 * (_Q // 12)[:, None] + _I[None, :] // 512)
           + ((_Q % 12) // 3)[:, None])
    + (_I[None, :] % 512)
)


def assemble(results):
    out = np.empty((N_CORES, PX_CORE, 3), np.float32)
    for n, r in enumerate(results):
        buf = np.asarray(r["out"], np.float32)
        out[n, _PX, np.broadcast_to(_C[:, None], _PX.shape)] = buf
    return out.reshape(H, W, 3)


def kernel(mean, alpha, scale, theta, rgb, pixels):
    global LAST_EXEC_NS, LAST_RESULTS
    in_maps = make_in_maps(mean, alpha, scale, theta, rgb, pixels)
    nc = _build_nc()
    trace = os.environ.get("KERNEL_TRACE", "0") == "1"
    res = run_bass_kernel_spmd(nc, in_maps, list(range(N_CORES)), trace=trace)
    LAST_EXEC_NS = res.exec_time_ns
    LAST_RESULTS = res
    return assemble(res.results)


# revision 38
# speedup vs baseline: 4.9797x; 1.3286x over previous
"""GaussianImage splat kernel for 8 Trainium2 NeuronCores.

Math (matches reference.py):
    cov      = R(theta) S S^T R(theta)^T per gaussian
    q(n, p)  = (p - mu_n)^T cov_n^{-1} (p - mu_n)
    prob     = exp(-q/2) / (2*pi*sqrt(det))
    splat    = prob / max(prob)           (global max over all n, p)
    img      = einsum('nhw,nc->hwc', splat, rgb*alpha)
    out      = sigmoid(img)

Device formulation:
    r(n, p)  = G[n] . F[p]   with F = [1, x', y', x'^2, x'y', y'^2] (centered
               monomials, x' = x - 0.5) and G the folded per-gaussian
               coefficients including log norm -> prob = exp(r).
    MM1 (PE, row-tiled 4x128-gaussian chunks), exp on ACT (PSUM->SBUF bf16),
    running elementwise max on DVE, MM2 (PE) contracts gaussians against
    w = rgb*alpha, 4 image tiles packed per PSUM bank at partition bases
    {0,32,64,96}, DVE evacuates, DMA repacks to pixel-major, AllReduce(max)
    over the 8 cores, then sigmoid via tanh (same ACT table set as exp).

Sharding: pixels split across the 8 cores (each core: all 512 gaussians x
32768 pixels); only a 512-byte all-reduce(max) collective is needed.
"""

import os
import sys
import numpy as np

for _p in ("/opt/trn_rl_repo", "/root/.axon_site/_ro/trn_rl_repo"):
    if os.path.isdir(_p) and _p not in sys.path:
        sys.path.insert(0, _p)

import ml_dtypes  # noqa: E402

def _ensure_ntff_hook():
    """antenv.axon_hooks is missing in this image; shim it and register the
    ctypes NTFF profiler from trn_agent_boot so trace=True works."""
    import types

    try:
        import antenv.axon_hooks  # noqa: F401
        return
    except ImportError:
        pass
    import antenv

    mod = types.ModuleType("antenv.axon_hooks")
    mod._hook = None

    def set_axon_ntff_profile_hook(h):
        mod._hook = h

    def get_axon_ntff_profile_hook():
        return mod._hook

    mod.set_axon_ntff_profile_hook = set_axon_ntff_profile_hook
    mod.get_axon_ntff_profile_hook = get_axon_ntff_profile_hook
    sys.modules["antenv.axon_hooks"] = mod
    antenv.axon_hooks = mod
    try:
        from trn_agent_boot.trn_boot import _ntff_profile_via_ctypes

        set_axon_ntff_profile_hook(
            _ntff_profile_via_ctypes("/opt/axon/libaxon_pjrt.so")
        )
    except Exception:
        pass


_ensure_ntff_hook()

import concourse.bass as bass  # noqa: E402
import concourse.tile as tile  # noqa: E402
from concourse import bass_isa, library_config, mybir  # noqa: E402
from concourse.tile import add_dep_helper  # noqa: E402
from concourse.bass_utils import run_bass_kernel_spmd  # noqa: E402

F32 = mybir.dt.float32
BF16 = mybir.dt.bfloat16
FP16 = mybir.dt.float16

N_GAUSS = 512
H = 512
W = 512
N_CORES = 8
PX_CORE = H * W // N_CORES      # 32768 pixels per core
PT = 512                        # pixels per tile
NT = PX_CORE // PT              # 64 tiles per core
NCHUNK = 4                      # gaussian chunks of 128

LAST_EXEC_NS = None
LAST_RESULTS = None


def _build_nc():
    nc = bass.Bass(num_devices=N_CORES)

    # Per-core inputs (host pre-sliced / pre-massaged).
    pix = nc.declare_dram_parameter("pix", [128, 512], F32, isOutput=False)
    gch = nc.declare_dram_parameter("gch", [128, 128], FP16, isOutput=False)
    gcl = nc.declare_dram_parameter("gcl", [128, 128], FP16, isOutput=False)
    kap = nc.declare_dram_parameter("kap", [128, 1], F32, isOutput=False)
    wts = nc.declare_dram_parameter("wts", [128, 128], BF16, isOutput=False)
    out = nc.declare_dram_parameter("out", [96, 1024], F32, isOutput=True)

    with tile.TileContext(nc) as tc:
        with (
            tc.tile_pool(name="const", bufs=1) as cpool,
            tc.tile_pool(name="rps", bufs=3, space="PSUM") as rpool,
            tc.tile_pool(name="ips", bufs=2, space="PSUM") as ipool,
            tc.tile_pool(name="prob", bufs=3) as ppool,
            tc.tile_pool(name="evac", bufs=3) as epool,
            tc.tile_pool(name="dram", bufs=1, space="DRAM") as dpool,
        ):
            # ---- load inputs --------------------------------------------
            t0 = cpool.tile([128, 512], F32, tag="t0")       # centered pixels
            nc.sync.dma_start(t0[:], pix[:])
            gh_sb = cpool.tile([128, 128], FP16, tag="gh")
            nc.sync.dma_start(gh_sb[:], gch[:])
            gl_sb = cpool.tile([128, 128], FP16, tag="gl")
            nc.sync.dma_start(gl_sb[:], gcl[:])
            kap_sb = cpool.tile([128, 1], F32, tag="kap")
            nc.sync.dma_start(kap_sb[:], kap[:])
            w_sb = cpool.tile([128, 128], BF16, tag="w")
            nc.sync.dma_start(w_sb[:], wts[:])

            # ---- feature planes [128, 6*256] ----------------------------
            # partition p holds pixels 256p..256p+255; t0 is (x', y')
            # interleaved so x' = t0[:, 0::2], y' = t0[:, 1::2].
            # One consolidated tile: all 6 writers are DVE -> one sem.
            xs = t0[:, 0:512:2]
            ys = t0[:, 1:512:2]
            planes = cpool.tile([128, 1536], F32, tag="planes")
            nc.vector.memset(planes[:, 0:256], 1.0)
            nc.vector.tensor_copy(planes[:, 256:512], xs)
            nc.vector.tensor_copy(planes[:, 512:768], ys)
            nc.vector.tensor_mul(planes[:, 768:1024], xs, xs)
            nc.vector.tensor_mul(planes[:, 1024:1280], xs, ys)
            nc.vector.tensor_mul(planes[:, 1280:1536], ys, ys)

            # scale by kappa (fits coefficients into fp16 range) and split
            # into exact fp16 hi + lo parts: f = (fh + fl) / kappa.
            nc.vector.tensor_scalar_mul(planes[:], planes[:], kap_sb[:])
            ph = cpool.tile([128, 1536], FP16, tag="ph")
            nc.vector.tensor_copy(ph[:], planes[:])
            pl = cpool.tile([128, 1536], FP16, tag="pl")
            nc.vector.tensor_sub(pl[:], planes[:], ph[:])

            # ---- F matrices [128, 32768] fp16: rows 32c+k = feature k ---
            # Bounce via DRAM so each fmat row-group has ONE writer DMA per
            # col-half (matmul sync-wait slots are scarce). DRAM layout is
            # feature-major [k=6][a=128][b=256]: contiguous 64KB runs.
            fmats = []
            for nm, pt_ in (("h", ph), ("l", pl)):
                fb = dpool.tile([128, 1536], FP16, tag="fb" + nm)
                fbf = fb[:].rearrange("a b -> (a b)")
                nc.sync.dma_start(
                    fbf.rearrange("(k a b) -> a k b", k=6, b=256), pt_[:]
                )
                fm = cpool.tile([128, PX_CORE], FP16, tag="fm" + nm)
                HLF = PX_CORE // 2
                for half in range(2):
                    for c in range(NCHUNK):
                        base = 32 * c
                        nc.sync.dma_start(
                            fm[base : base + 6, HLF * half : HLF * (half + 1)],
                            fbf.rearrange("(k h ab) -> k h ab", k=6, h=2)[
                                :, half, :
                            ],
                        )
                fmats.append(fm)
            fmat_h, fmat_l = fmats

            # ---- persistent accumulators --------------------------------
            maxbuf = cpool.tile([128, 2048], BF16, tag="maxb")
            nc.vector.memset(maxbuf[:], 0.0)
            # channel-major image staging in DRAM: row 32j+c holds channel c
            # of tiles t%4==j; col 512u+i = pixel i of tile 4u+j.
            imgdram = dpool.tile([128, 8192], F32, tag="imgd")

            # ---- main loop (software-pipelined: consume prob one
            # tile late so PE's next MM1 isn't queued behind MM2) ---------
            probs = [None] * NT

            def consume(t):
                """MM2 + running max + evacuation for tile t."""
                prob = probs[t]
                j = t % 4
                u = t // 4
                nc.vector.tensor_tensor(
                    maxbuf[:], maxbuf[:], prob[:], op=mybir.AluOpType.max
                )
                if j == 0:
                    consume.imgp = ipool.tile([128, 512], F32, tag="img")
                imgp = consume.imgp
                for c in range(NCHUNK):
                    # M padded to 32 (cols 3..31 of wts are zero) so the
                    # whole 32-partition group is written -> defined PSUM.
                    nc.tensor.matmul(
                        imgp[32 * j : 32 * j + 32, :],
                        lhsT=w_sb[:, 32 * c : 32 * c + 32],
                        rhs=prob[:, 512 * c : 512 * (c + 1)],
                        start=(c == 0),
                        stop=(c == 3),
                        tile_position=(0, 32 * j),
                    )
                if j == 3:
                    ev = epool.tile([128, 512], F32, tag="ev")
                    nc.vector.tensor_copy(ev[:], imgp[:])
                    nc.sync.dma_start(imgdram[:, 512 * u : 512 * (u + 1)], ev[:])

            for t in range(NT):
                # MM1: r = G . F via 3 exact-product fp16 matmul passes
                # (gh@fh + gh@fl + gl@fh), row-tiled over 4 gaussian chunks.
                r0 = rpool.tile([128, 1024], F32, tag="r")
                r1 = rpool.tile([128, 1024], F32, tag="r")
                rhalves = (r0, r1)
                for p, (gg, ff) in enumerate(
                    ((gh_sb, fmat_h), (gh_sb, fmat_l), (gl_sb, fmat_h))
                ):
                    for c in range(NCHUNK):
                        rh = rhalves[c // 2]
                        off = 512 * (c % 2)
                        nc.tensor.matmul(
                            rh[:, off : off + 512],
                            lhsT=gg[32 * c : 32 * c + 6, :],
                            rhs=ff[32 * c : 32 * c + 6, PT * t : PT * (t + 1)],
                            start=(p == 0),
                            stop=(p == 2),
                            tile_position=(32 * c, 0),
                        )

                # exp on ACT: PSUM fp32 -> SBUF bf16
                prob = ppool.tile([128, 2048], BF16, tag="prob")
                nc.scalar.activation(
                    prob[:, 0:1024], r0[:], mybir.ActivationFunctionType.Exp
                )
                nc.scalar.activation(
                    prob[:, 1024:2048], r1[:], mybir.ActivationFunctionType.Exp
                )
                probs[t] = prob

                if t >= 1:
                    consume(t - 1)
            consume(NT - 1)

            # ---- global max ---------------------------------------------
            # per-partition max -> straight into the collective buffer; the
            # AllReduce(max) reduces across cores elementwise, the scalar
            # reduce of the 128-vector happens after.
            maxcol = cpool.tile([128, 1], F32, tag="maxcol")
            nc.vector.tensor_reduce(
                maxcol[:],
                maxbuf[:],
                axis=mybir.AxisListType.X,
                op=mybir.AluOpType.max,
            )
            cc_in = dpool.tile([1, 128], F32, tag="ccin")
            cc_out = dpool.tile([1, 128], F32, tag="ccout")
            nc.sync.dma_start(
                cc_in[:].rearrange("a b -> (a b)"),
                maxcol[:].rearrange("a b -> (a b)"),
            )
            nc.gpsimd.collective_compute(
                "AllReduce",
                mybir.AluOpType.max,
                replica_groups=[list(range(N_CORES))],
                ins=[cc_in[:].opt()],
                outs=[cc_out[:].opt()],
            )
            row = cpool.tile([1, 128], F32, tag="row")
            nc.sync.dma_start(row[:], cc_out[:])
            ones_row = cpool.tile([1, 128], F32, tag="onesrow")
            nc.vector.memset(ones_row[:], 1.0)
            pmax = cpool.tile([1, 1], F32, tag="pmax")
            nc.vector.tensor_reduce(
                pmax[:], row[:], axis=mybir.AxisListType.X, op=mybir.AluOpType.max
            )

            # s = 1 / (2 * pmax), broadcast to all 128 partitions via DRAM
            srec = cpool.tile([1, 1], F32, tag="srec")
            nc.vector.tensor_scalar_mul(srec[:], pmax[:], 2.0)
            nc.vector.reciprocal(srec[:], srec[:])
            srow = cpool.tile([1, 128], F32, tag="srow")
            nc.vector.tensor_scalar_mul(srow[:], ones_row[:], srec[:])
            d2 = dpool.tile([1, 128], F32, tag="d2")
            nc.sync.dma_start(d2[:], srow[:])
            sbc = cpool.tile([128, 1], F32, tag="sbc")
            nc.sync.dma_start(
                sbc[:].rearrange("a b -> (a b)"),
                d2[:].rearrange("a b -> (a b)"),
            )

            # ---- compact the 12 real rows of each slab to [96, 1024] ----
            # partition 12g+3j+c, col 512h+i  <-  imgdram[32j+c, 1024g+512h+i]
            fin = cpool.tile([96, 1024], F32, tag="fin")
            for g in range(8):
                for j in range(4):
                    nc.sync.dma_start(
                        fin[12 * g + 3 * j : 12 * g + 3 * j + 3, :],
                        imgdram[32 * j : 32 * j + 3, 1024 * g : 1024 * (g + 1)],
                    )

            # ---- final pass: sigmoid(acc/pmax) = 0.5 + 0.5*tanh(acc*s) --
            nc.vector.tensor_scalar_mul(fin[:], fin[:], sbc[0:96, :])
            nc.scalar.activation(
                fin[:], fin[:], mybir.ActivationFunctionType.Tanh
            )
            nc.vector.tensor_scalar(
                fin[:],
                fin[:],
                0.5,
                0.5,
                op0=mybir.AluOpType.mult,
                op1=mybir.AluOpType.add,
            )
            nc.sync.dma_start(out[:], fin[:])

    _legalize_waits(nc)
    return nc


# walrus encodes sync waits into fixed ISA struct slots (fused matmuls /
# TT hold only ONE). Hoist excess waits onto same-engine NOPs spliced
# immediately before the instruction — semantically identical (the engine
# stalls at the NOP instead).
def _legalize_waits(nc, cap=1):
    for blk in nc.main_func.blocks:
        insts = blk.instructions
        out = []
        for ins in insts:
            si = ins.sync_info
            if si is not None and len(si.on_wait) > cap:
                waits = list(si.on_wait)
                excess, keep = waits[:-cap], waits[-cap:]
                for w in excess:
                    eng = nc.engines[ins.engine]
                    n = eng.nop(hint="wait_legalize")
                    tail = nc.main_func.blocks[-1].instructions
                    assert tail[-1] is n.ins
                    tail.pop()
                    n.ins.sync_info = mybir.SyncInfo(
                        on_wait=[w], on_update=[]
                    )
                    out.append(n.ins)
                si.on_wait = keep
            out.append(ins)
        insts[:] = out


def _host_prep(mean, alpha, scale, theta, rgb, pixels):
    """Fold gaussian params into matmul coefficients (float64 on host)."""
    mean = np.asarray(mean, np.float64)
    alpha = np.asarray(alpha, np.float64)
    scale = np.asarray(scale, np.float64)
    theta = np.asarray(theta, np.float64)
    rgb = np.asarray(rgb, np.float64)
    pixels = np.asarray(pixels, np.float32)

    two_pi = 2.0 * np.pi
    ta = two_pi * theta[:, 0]
    c, s = np.cos(ta), np.sin(ta)
    sx2 = scale[:, 0] ** 2
    sy2 = scale[:, 1] ** 2
    A = c * c * sx2 + s * s * sy2
    Bc = c * s * (sx2 - sy2)
    D = s * s * sx2 + c * c * sy2
    det = A * D - Bc * Bc
    i00 = D / det
    iBs = -2.0 * Bc / det          # inv01 + inv10
    i11 = A / det
    lognorm = -np.log(two_pi) - 0.5 * np.log(det)

    px0 = mean[:, 0, 0] - 0.5      # gaussian means in centered coords
    py0 = mean[:, 1, 0] - 0.5
    c_1 = (
        -0.5 * (i00 * px0 * px0 + iBs * px0 * py0 + i11 * py0 * py0) + lognorm
    )
    c_x = i00 * px0 + 0.5 * iBs * py0
    c_y = 0.5 * iBs * px0 + i11 * py0
    c_xx = -0.5 * i00
    c_xy = -0.5 * iBs
    c_yy = -0.5 * i11

    # scale coefficients into fp16 range; features get multiplied by
    # kappa on device so c'*f' == c*f exactly.
    coefs = np.stack([c_1, c_x, c_y, c_xx, c_xy, c_yy])  # [6, 512]
    cmax = np.abs(coefs).max()
    kappa = max(1.0, cmax / 16384.0)
    cs = coefs / kappa
    ch = cs.astype(np.float16)
    cl = (cs - ch.astype(np.float64)).astype(np.float16)
    gch = np.zeros((128, 128), np.float16)
    gcl = np.zeros((128, 128), np.float16)
    for chk in range(NCHUNK):
        gch[32 * chk : 32 * chk + 6, :] = ch[:, 128 * chk : 128 * (chk + 1)]
        gcl[32 * chk : 32 * chk + 6, :] = cl[:, 128 * chk : 128 * (chk + 1)]
    kap = np.full((128, 1), kappa, np.float32)

    w = (rgb * alpha).astype(ml_dtypes.bfloat16)          # [512, 3]
    wts = np.zeros((128, 128), ml_dtypes.bfloat16)
    for ch in range(NCHUNK):
        wts[:, 32 * ch : 32 * ch + 3] = w[128 * ch : 128 * (ch + 1), :]

    # centered pixels, per-core slices in [128, 512] partition layout
    pc = (pixels.astype(np.float32) - np.float32(0.5)).reshape(-1)  # (H*W*2,)
    pix_cores = pc.reshape(N_CORES, 128, 512)
    return gch, gcl, kap, wts, pix_cores


def make_in_maps(mean, alpha, scale, theta, rgb, pixels):
    gch, gcl, kap, wts, pix_cores = _host_prep(mean, alpha, scale, theta, rgb, pixels)
    return [
        {"pix": np.ascontiguousarray(pix_cores[i]), "gch": gch, "gcl": gcl,
         "kap": kap, "wts": wts}
        for i in range(N_CORES)
    ]


_Q = np.arange(96)
_I = np.arange(1024)
_C = _Q % 3
_PX = (
    512 * (4 * (2 * (_Q // 12)[:, None] + _I[None, :] // 512)
           + ((_Q % 12) // 3)[:, None])
    + (_I[None, :] % 512)
)


def assemble(results):
    out = np.empty((N_CORES, PX_CORE, 3), np.float32)
    for n, r in enumerate(results):
        buf = np.asarray(r["out"], np.float32)
        out[n, _PX, np.broadcast_to(_C[:, None], _PX.shape)] = buf
    return out.reshape(H, W, 3)


def kernel(mean, alpha, scale, theta, rgb, pixels):
    global LAST_EXEC_NS, LAST_RESULTS
    in_maps = make_in_maps(mean, alpha, scale, theta, rgb, pixels)
    nc = _build_nc()
    trace = os.environ.get("KERNEL_TRACE", "0") == "1"
    res = run_bass_kernel_spmd(nc, in_maps, list(range(N_CORES)), trace=trace)
    LAST_EXEC_NS = res.exec_time_ns
    LAST_RESULTS = res
    return assemble(res.results)
